# revision 4
# baseline (speedup 1.0000x reference)
"""DeepReservoir (leaky ESN, 4 modules) Trainium2 Bass kernel.

Problem: h[t] = (1-a)*h[t-1] + a*tanh(u[t] @ Kin + h[t-1] @ W + bias) per
module, T=8192 steps, U=1024 units, a=0.9. Output = all states, modules
concatenated on the feature axis: [1, T, 4*1024].

Strategy (module parallel, per the sharding hint):
  - One reservoir module per NeuronCore (4 modules; cores 4-7 run duplicates
    so one SPMD program serves all 8 cores; host gathers from cores 0-3).
  - The input projection c[t] = u[t] @ Kin + bias has no time dependence:
    precomputed on TensorE into a DRAM scratch, streamed back per chunk.
  - The time scan is the serial bottleneck: per step a [1024]x[1024,1024]
    matvec on TensorE (64 LDWEIGHTS+MATMUL pairs of [128,128]x[128,1]),
    weight-load bound.  Weights are bf16 (enables fast-weight-load) with
    leaky a folded in: W' = a*W.  State kept in fp32 via the rescaled
    recurrence h'[t] = (1-a)*h'[t-1] + tanh(W' h'[t-1] + c[t]); the output
    is a*h'.
  - tanh runs on ScalarE reading PSUM directly with c[t] as the per-partition
    activation bias; blend/cast on VectorE.  Matmuls are phase-ordered so
    ScalarE/VectorE work on the first half of the state while TensorE
    finishes the second half -> TensorE stays busy.
  - Time loop: tc.For_i with UNROLL steps per iteration.
"""

import numpy as np
import ml_dtypes

import concourse.bacc as bacc
import concourse.bass as bass
import concourse.tile as tile
import concourse.mybir as mybir
from concourse.bass import ds
from concourse.bass_utils import run_bass_kernel_spmd

F32 = mybir.dt.float32
BF16 = mybir.dt.bfloat16

UNITS = 1024
IN = 64
KT = 8  # contraction tiles (1024/128)
MT = 8  # output tiles (1024/128)
P = 128

LEAKY = np.float32(0.9)
ONE_MINUS_LEAKY = float(np.float32(1.0) - np.float32(0.9))

N_CORES = 8
N_MODULES = 4


def build_nc(T: int, unroll: int):
    """Build the single-core SPMD Bass program for one reservoir module."""
    assert T % unroll == 0 and unroll % 2 == 0
    nc = bacc.Bacc("TRN2", debug=False)

    wT = nc.dram_tensor("wT", [UNITS, UNITS], BF16, kind="ExternalInput")
    uT = nc.dram_tensor("uT", [IN, T], F32, kind="ExternalInput")
    kin = nc.dram_tensor("kin", [IN, UNITS], F32, kind="ExternalInput")
    bias_pj = nc.dram_tensor("bias_pj", [P, MT], F32, kind="ExternalInput")
    hs = nc.dram_tensor("hs", [T, UNITS], F32, kind="ExternalOutput")
    c_dram = nc.dram_tensor("c_scratch", [T, MT, P], F32, kind="Internal")

    with tile.TileContext(nc) as tc:
        with (
            tc.tile_pool(name="const", bufs=1) as const_pool,
            tc.tile_pool(name="pre", bufs=3) as pre_pool,
            tc.tile_pool(name="prepsum", bufs=2, space="PSUM") as prepsum_pool,
            tc.tile_pool(name="cin", bufs=2) as cin_pool,
            tc.tile_pool(name="hout", bufs=2) as hout_pool,
            tc.tile_pool(name="work", bufs=2) as work_pool,
            tc.tile_pool(name="zpsum", bufs=2, space="PSUM") as zpsum_pool,
        ):
            # ---- resident tensors -------------------------------------
            # weights: w_sb[p, k, m, c] = W'[k*128+p, m*128+c]
            w_sb = const_pool.tile([P, KT, MT, P], BF16)
            nc.sync.dma_start(
                w_sb[:],
                wT[:, :].rearrange("(k p) (m c) -> p k m c", p=P, c=P),
            )
            u_sb = const_pool.tile([IN, T], F32)
            nc.sync.dma_start(u_sb[:], uT[:, :])
            kin_sb = const_pool.tile([IN, UNITS], F32)
            nc.sync.dma_start(kin_sb[:], kin[:, :])
            bias_sb = const_pool.tile([P, MT], F32)
            nc.sync.dma_start(bias_sb[:], bias_pj[:, :])

            # persistent scan state (ping-pong on dim 1 by step parity)
            hstate = const_pool.tile([P, 2, MT], F32)  # h' fp32 master
            h16 = const_pool.tile([P, 2, MT], BF16)  # bf16 copy for PE rhs
            nc.vector.memset(hstate[:, 1, :], 0.0)
            nc.vector.memset(h16[:, 1, :], 0.0)

            # ---- precompute c[t, u] = (u @ Kin)[t, u] + bias[u] -------
            c_w = c_dram[:, :, :].rearrange("t j p -> p j t")
            tchunk = 512
            for j in range(MT):
                for t0 in range(0, T, tchunk):
                    n = min(tchunk, T - t0)
                    cp = prepsum_pool.tile([P, tchunk], F32, tag="cp")
                    nc.tensor.matmul(
                        cp[:, :n],
                        kin_sb[:, j * P : (j + 1) * P],
                        u_sb[:, t0 : t0 + n],
                        start=True,
                        stop=True,
                    )
                    cstage = pre_pool.tile([P, tchunk], F32, tag="cstage")
                    nc.vector.tensor_scalar_add(
                        cstage[:, :n], cp[:, :n], bias_sb[:, j : j + 1]
                    )
                    nc.sync.dma_start(c_w[:, j, t0 : t0 + n], cstage[:, :n])

            # ---- time scan --------------------------------------------
            c_r = c_dram[:, :, :].rearrange("t j p -> p t j")
            hs_w = hs[:, :].rearrange("t (j p) -> p t j", p=P)

            with tc.For_i(
                0,
                T,
                unroll,
                hint_engines=(mybir.EngineType.PE, mybir.EngineType.Activation),
            ) as iv:
                cchunk = cin_pool.tile([P, unroll, MT], F32, tag="cchunk")
                nc.sync.dma_start(cchunk[:], c_r[:, ds(iv, unroll), :])
                hstage = hout_pool.tile([P, unroll, MT], F32, tag="hstage")

                for s in range(unroll):
                    cur = s % 2
                    prev = 1 - cur
                    zA = zpsum_pool.tile([P, 4], F32, tag="zA")
                    zB = zpsum_pool.tile([P, 4], F32, tag="zB")

                    def mm(k, m, start, stop):
                        zt = zA if m < 4 else zB
                        nc.tensor.matmul(
                            zt[:, (m % 4) : (m % 4) + 1],
                            w_sb[:, k, m, :],
                            h16[:, prev, k : k + 1],
                            start=start,
                            stop=stop,
                        )

                    # phase 1: contraction tiles 0-3 (need only half A of
                    # h16, which the previous step produced early)
                    for k in range(4):
                        for m in range(MT):
                            mm(k, m, start=(k == 0 and m % 4 == 0), stop=False)
                    # phase 2a: finish columns 0-3 so ScalarE can start
                    for m in range(4):
                        for k in range(4, 8):
                            mm(k, m, start=False, stop=(k == 7 and m == 3))
                    # phase 2b: finish columns 4-7
                    for m in range(4, 8):
                        for k in range(4, 8):
                            mm(k, m, start=False, stop=(k == 7 and m == 7))

                    o32 = work_pool.tile([P, MT], F32, tag="o32")
                    for j in range(MT):
                        zt = zA if j < 4 else zB
                        nc.scalar.activation(
                            o32[:, j : j + 1],
                            zt[:, (j % 4) : (j % 4) + 1],
                            mybir.ActivationFunctionType.Tanh,
                            bias=cchunk[:, s, j : j + 1],
                        )
                    for lo, hi in ((0, 4), (4, 8)):
                        # h'[t] = (1-a)*h'[t-1] + tanh(...)
                        nc.vector.scalar_tensor_tensor(
                            out=hstate[:, cur, lo:hi],
                            in0=hstate[:, prev, lo:hi],
                            scalar=ONE_MINUS_LEAKY,
                            in1=o32[:, lo:hi],
                            op0=mybir.AluOpType.mult,
                            op1=mybir.AluOpType.add,
                        )
                        nc.vector.tensor_copy(
                            h16[:, cur, lo:hi], hstate[:, cur, lo:hi]
                        )
                    # output h[t] = a * h'[t]
                    nc.vector.tensor_scalar_mul(
                        hstage[:, s, :], hstate[:, cur, :], float(LEAKY)
                    )

                nc.sync.dma_start(hs_w[:, ds(iv, unroll), :], hstage[:])

    nc.compile()
    return nc


def _prep_in_maps(u, kernel, rec_kernel, bias, T):
    uT = np.ascontiguousarray(u[0].T).astype(np.float32)  # [64, T]
    in_maps = []
    for c in range(N_CORES):
        m = c % N_MODULES
        wT = np.ascontiguousarray(
            (rec_kernel[m].astype(np.float32) * LEAKY).astype(ml_dtypes.bfloat16)
        )
        in_maps.append(
            {
                "wT": wT,
                "uT": uT,
                "kin": np.ascontiguousarray(kernel[m].astype(np.float32)),
                "bias_pj": np.ascontiguousarray(
                    bias[m].astype(np.float32).reshape(MT, P).T
                ),
            }
        )
    return in_maps


_NC_CACHE = {}


def run(u, kernel, rec_kernel, bias, unroll=32, trace=False):
    T = u.shape[1]
    key = (T, unroll)
    if key not in _NC_CACHE:
        _NC_CACHE[key] = build_nc(T, unroll)
    nc = _NC_CACHE[key]
    in_maps = _prep_in_maps(u, kernel, rec_kernel, bias, T)
    res = run_bass_kernel_spmd(
        nc, in_maps, core_ids=list(range(N_CORES)), trace=trace
    )
    out = np.concatenate(
        [res.results[m]["hs"] for m in range(N_MODULES)], axis=1
    )  # [T, 4096]
    return out[None].astype(np.float32), res


def kernel(u, kernel, rec_kernel, bias):
    out, _ = run(u, kernel, rec_kernel, bias)
    return out
